# revision 22
# baseline (speedup 1.0000x reference)
"""Trainium2 Bass kernel for a single attention head (v3: no collectives).

reference computation (fp32):
    q = query @ Wq + bq ; k = key @ Wk + bk ; v = value @ Wv + bv
    out = softmax((q @ k^T) / 8) @ v

Sharding: 8 cores, core c -> (batch b = c//2, query-half h = c%2). Each core
loads its q half transposed [512, 2048] plus the FULL k^T/v^T of its batch
[512, 4096] -- all host-pre-transposed and host-cast to bf16 (pure layout
prep; all projections/attention FLOPs stay on device). 10 MiB per core, no
inter-core exchange at all (the v2 pair-AllGather design lost ~50 us to
collective launch latency), and no PE input transposes (x^T comes in the
contraction-major layout the projection matmuls want).

bk is dropped entirely: softmax is invariant to per-query constants.

Per-core dataflow (bf16 matmuls, fp32 PSUM):
  - x^T loads chunked on three DMA queues (Act: k, DVE: q, SWDGE: v) so
    projections start as soon as the first chunks land
  - projections: lhsT = W [c-chunk, d] bf16, rhs = x^T -> Qp^T/Kp^T [64, s]
    Qp^T duplicated to partitions 64:128, Kp^T dual-half (even 128-key chunks
    on partitions 0:64, odd on 64:128 via partition-shift DMA) so the scores
    matmuls can row-tile across PE quadrants; V^T projected then PE-transposed
    to natural [keys, 66] with col 64 = ones (softmax denominator via the PV
    matmul), col 65 zero pad
  - scores^T tiles: lhsT = Kp^T[half, chunk] [64,128], rhs = Qp^T [64, 1024];
    exp fused with the 1/8 scale: half 0 on ScalarE (exact exp), half 1 on
    DVE (Schraudolph bf16 bit-trick) so the two engines split the exp stream
  - PV: lhsT = v[chunk] [128, 66] bf16, rhs = P^T, accumulated in PSUM ->
    out^T [66, 1024] (row 64 = denominator) over all 32 key chunks
  - epilogue: copy to SBUF, PE-transpose out^T, reciprocal + scale, DMA out.
"""

import sys

if "/opt/trn_rl_repo" not in sys.path:
    sys.path.insert(0, "/opt/trn_rl_repo")

from contextlib import ExitStack

import numpy as np
import ml_dtypes

import concourse.bass as bass
import concourse.tile as tile
from concourse import bacc, mybir
from concourse.bass_utils import run_bass_kernel_spmd
from concourse.masks import make_identity

F32 = mybir.dt.float32
F32R = mybir.dt.float32r
BF = mybir.dt.bfloat16
FP8 = mybir.dt.float8e4
DR = mybir.MatmulPerfMode.DoubleRow
BF_NP = ml_dtypes.bfloat16
B, S, C, D = 4, 4096, 512, 64
D2 = D + 2          # v padded with [ones, zeros] cols
N_CORES = 8
SQ = S // 2          # query rows per core
SK = S               # key rows per core (full batch)
NJ = SK // 128       # 32 key chunks of 128 rows
NP = NJ // 2         # 16 j-pairs
IH = SQ // 2         # 1024: i-half processed per PSUM residency
ST_W = 1024
EXP = mybir.ActivationFunctionType.Exp
CPY = mybir.ActivationFunctionType.Copy
MUL = mybir.AluOpType.mult
ADD = mybir.AluOpType.add

_CACHE = {}

# Schraudolph bf16 exp on DVE: bits(exp(s/8)) ~= round(s*A + B) as int16,
# reinterpreted as bf16 (7 mantissa bits, bias 127). A = 2^7*log2(e)/8;
# B = 127*2^7 - 0.045*2^7 centers the piecewise-linear-mantissa error
# (~+-3% max on the weights; softmax averaging over ~2k keys shrinks it
# far below budget).
SCH_A = 128.0 * 1.4426950408889634 / 8.0
SCH_B = 127.0 * 128.0 - 0.045 * 128.0
I16 = mybir.dt.int16


def _emit(nc, tc, aps):
    qt_d, kt_d, vt_d, wq_d, wk_d, wvp_d, bq_d, bvp_d, out_d = aps

    ctx = ExitStack()
    const = ctx.enter_context(tc.tile_pool(name="const", bufs=1))
    persist = ctx.enter_context(tc.tile_pool(name="persist", bufs=1))
    pt_p = ctx.enter_context(tc.tile_pool(name="pt", bufs=48))
    ep_p = ctx.enter_context(tc.tile_pool(name="ep", bufs=2))
    small_p = ctx.enter_context(tc.tile_pool(name="small", bufs=4))
    out_p = ctx.enter_context(tc.tile_pool(name="outp", bufs=2))
    st_ps = ctx.enter_context(tc.tile_pool(name="stps", bufs=4, space="PSUM"))
    po_ps = ctx.enter_context(tc.tile_pool(name="pops", bufs=1, space="PSUM"))
    ms_ps = ctx.enter_context(tc.tile_pool(name="msps", bufs=2, space="PSUM"))

    ident32 = const.tile([128, 128], F32)
    make_identity(nc, ident32[:])
    identb = const.tile([128, 128], BF)
    nc.vector.tensor_copy(identb[:], ident32[:])
    identr = const.tile([128, 128], F32R)
    nc.vector.tensor_copy(identr[:], ident32[:])

    # weights via SP HWDGE (fp32) + DVE round to bf16
    wq32 = const.tile([128, 4, D], F32)
    nc.sync.dma_start(wq32[:], wq_d.rearrange("(cc p) d -> p cc d", p=128))
    wq_sb = const.tile([128, 4, D], BF)
    nc.vector.tensor_copy(wq_sb[:], wq32[:])
    wk32 = const.tile([128, 4, D], F32)
    nc.sync.dma_start(wk32[:], wk_d.rearrange("(cc p) d -> p cc d", p=128))
    wk_sb = const.tile([128, 4, D], BF)
    nc.vector.tensor_copy(wk_sb[:], wk32[:])
    wvp32 = const.tile([128, 4, D2], F32)
    nc.sync.dma_start(wvp32[:], wvp_d.rearrange("(cc p) d -> p cc d", p=128))
    wvp_sb = const.tile([128, 4, D2], BF)
    nc.vector.tensor_copy(wvp_sb[:], wvp32[:])
    bq_sb = const.tile([D, 1], F32)
    nc.sync.dma_start(bq_sb[:], bq_d[:])
    bvp_sb = const.tile([D2, 1], F32)
    nc.sync.dma_start(bvp_sb[:], bvp_d[:])

    qts = persist.tile([128, 4, SQ], BF)   # q^T staged (c on partitions)
    kts = persist.tile([128, 4, SK], BF)   # k^T staged
    vts = persist.tile([128, 4, SK], BF)   # v^T staged
    # fp8 Qp^T / Kp^T for DoubleRow scores matmuls (0.5 cyc/row). kx8's
    # second k-tile is zeroed once; the rhs broadcasts Qp^T over both
    # k-tiles with a stride-0 dim, so tile 1 contributes w1^T@q = 0.
    qp8 = persist.tile([D, SQ], FP8)
    kx8 = persist.tile([D, 2, SK], FP8)
    vx = persist.tile([128, NJ, D2], BF)   # v natural + ones col
    nc.gpsimd.memset(kx8[:, 1, :], 0.0)

    # ---- staged loads: 512-col chunks, dependency-ordered ---------------
    # Act queue: k^T (8 chunks); SP queue: q^T (4); SWDGE (gpsimd): v^T (8).
    # Issue order approximates the order the PE stream consumes them; the
    # DMA engine pool drains roughly in issue order.
    ktv = kt_d.rearrange("(cc p) s -> p cc s", p=128)
    qtv = qt_d.rearrange("(cc p) s -> p cc s", p=128)
    vtv = vt_d.rearrange("(cc p) s -> p cc s", p=128)

    def ch(x, g):
        return x[:, :, g * 512 : (g + 1) * 512]

    for t, g in [("k", 0), ("k", 1), ("q", 0), ("q", 1), ("v", 0), ("v", 1),
                 ("k", 2), ("v", 2), ("k", 3), ("k", 4), ("v", 3), ("k", 5),
                 ("k", 6), ("k", 7), ("v", 4), ("q", 2), ("q", 3), ("v", 5),
                 ("v", 6), ("v", 7)]:
        if t == "k":
            nc.scalar.dma_start(ch(kts, g), ch(ktv, g))
        elif t == "q":
            nc.sync.dma_start(ch(qts, g), ch(qtv, g))
        else:
            nc.gpsimd.dma_start(ch(vts, g), ch(vtv, g))

    # ---- projections ----------------------------------------------------
    def proj(xts, w_sb, m, g, sink):
        """Project one 512-col group: pp [m, 512] PSUM; sink consumes it."""
        pp = ms_ps.tile([D2, 512], F32, tag="ms")
        for cc in range(4):
            nc.tensor.matmul(
                pp[:m, :], w_sb[:, cc, :m], xts[:, cc, g * 512 : (g + 1) * 512],
                start=(cc == 0), stop=(cc == 3),
            )
        sink(pp)

    # bias-free PSUM drains run on ScalarE (Act Copy needs no act table, so
    # no table thrash with Exp); biased sinks stay on DVE
    def sink_q(g):
        def f(pp):
            sl = slice(g * 512, (g + 1) * 512)
            nc.vector.tensor_scalar_add(qp8[:, sl], pp[:D, :], bq_sb[:])
        return f

    def sink_k(g):
        # pp [64, 512] = key rows g*512..(g+1)*512 (no bias: bk dropped)
        def f(pp):
            nc.scalar.activation(kx8[:, 0, g * 512 : (g + 1) * 512], pp[:D, :], CPY)
        return f

    def sink_v(g):
        def f(pp):
            vt = ep_p.tile([D2, 512], BF, tag="vt")
            nc.vector.tensor_scalar_add(vt[:], pp[:, :], bvp_sb[:])
            # 4 transposes batched into one PSUM tile -> single drain
            vnp = ms_ps.tile([128, 4, D2], BF, tag="ms")
            for r in range(4):
                nc.tensor.transpose(
                    vnp[:, r, :], vt[:, r * 128 : (r + 1) * 128], identb[:D2, :D2]
                )
            nc.vector.tensor_copy(vx[:, g * 4 : g * 4 + 4, :], vnp[:])
        return f

    # ---- attention helpers ----------------------------------------------
    def scores_exp(lp, ih):
        """Scores + exp for one j-pair against i-half ih; returns 4 P^T bf16
        APs indexed [half*2+n] ([128, 512] each). The exp stream alternates
        between ScalarE (exact exp) and DVE (Schraudolph bit-trick); st tiles
        are a single PSUM bank each so 4 ring slots fit alongside po."""
        sts = []
        for half in range(2):
            j = 2 * lp + half
            for n in range(ST_W // 512):
                st = st_ps.tile([128, 512], F32, tag="st")
                nc.tensor.matmul(
                    st[:],
                    kx8[:, :, j * 128 : (j + 1) * 128],
                    qp8[:, ih * IH + n * 512 : ih * IH + (n + 1) * 512]
                    .unsqueeze(1).broadcast_to([D, 2, 512]),
                    perf_mode=DR,
                )
                idx = half * 2 + n
                on_act = (idx + lp) % 2 == 0 or (lp % 4 == 3 and idx == 1)
                if on_act:
                    pt = pt_p.tile([128, 512], BF, tag="pt")
                    nc.scalar.activation(pt[:], st[:], EXP, scale=0.125)
                    sts.append(pt[:])
                else:
                    pt16 = pt_p.tile([128, 512], I16, tag="pt")
                    nc.vector.tensor_scalar(pt16[:], st[:], SCH_A, SCH_B, MUL, ADD)
                    sts.append(pt16[:].bitcast(BF))
        return sts

    def pv(lp, po, sts, first, last):
        for half in range(2):
            for n in range(ST_W // 512):
                nc.tensor.matmul(
                    po[:, n * 512 : (n + 1) * 512],
                    vx[:, 2 * lp + half, :],
                    sts[half * 2 + n],
                    start=(first and half == 0), stop=(last and half == 1),
                )

    def epilogue(ih, po):
        ot = ep_p.tile([D2, IH], F32R, tag="ot")
        nc.scalar.activation(ot[:], po[:], CPY)
        osb = out_p.tile([128, IH // 128, D], F32, tag="osb")
        for b in range(IH // 512):
            onat = ms_ps.tile([128, 4, D2], F32R, tag="ms")
            for r in range(4):
                t = b * 4 + r
                nc.tensor.transpose(
                    onat[:, r, :], ot[:, t * 128 : (t + 1) * 128], identr[:D2, :D2]
                )
            for r in range(4):
                t = b * 4 + r
                rs = small_p.tile([128, 1], F32, tag="rs")
                nc.vector.reciprocal(rs[:], onat[:, r, D : D + 1])
                nc.vector.tensor_scalar_mul(osb[:, t, :], onat[:, r, :D], rs[:])
        nc.sync.dma_start(
            out_d[ih * IH : (ih + 1) * IH, :].rearrange("(t p) d -> p t d", p=128),
            osb[:],
        )

    # ---- schedule -------------------------------------------------------
    # Minimal prologue (kproj 0-1 + qproj 0-1 unlock scores lp 0-3), then the
    # ih0 attention stream with the remaining k/q/v projections interleaved
    # just ahead of the matmuls that consume them. PV is software-pipelined
    # LAG j-pairs behind scores (the pt ring carries the in-flight P^T tiles)
    # so a late v chunk never head-of-line-blocks the PE queue.
    LAG = 5
    proj(kts, wk_sb, D, 0, sink_k(0))
    proj(kts, wk_sb, D, 1, sink_k(1))
    proj(qts, wq_sb, D, 0, sink_q(0))
    proj(qts, wq_sb, D, 1, sink_q(1))

    for ih in range(2):
        po = po_ps.tile([D2, IH], F32, tag="po")
        pend = []
        for lp in range(NP):
            if ih == 0:
                if lp % 2 == 0:
                    g = lp // 2
                    if g + 2 < 8:
                        proj(kts, wk_sb, D, g + 2, sink_k(g + 2))
                    proj(vts, wvp_sb, D2, g, sink_v(g))
                elif lp in (11, 13):
                    proj(qts, wq_sb, D, (lp - 7) // 2, sink_q((lp - 7) // 2))
            pend.append((lp, scores_exp(lp, ih)))
            if len(pend) > LAG:
                l0, s0 = pend.pop(0)
                pv(l0, po, s0, first=(l0 == 0), last=False)
        for l0, s0 in pend:
            pv(l0, po, s0, first=(l0 == 0), last=(l0 == NP - 1))
        epilogue(ih, po)
    ctx.close()


def _build(reps=1):
    nc = bacc.Bacc("TRN2", target_bir_lowering=False, debug=False, num_devices=N_CORES)
    aps = (
        nc.dram_tensor("qt", [C, SQ], BF, kind="ExternalInput").ap(),
        nc.dram_tensor("kt", [C, SK], BF, kind="ExternalInput").ap(),
        nc.dram_tensor("vt", [C, SK], BF, kind="ExternalInput").ap(),
        nc.dram_tensor("wq", [C, D], F32, kind="ExternalInput").ap(),
        nc.dram_tensor("wk", [C, D], F32, kind="ExternalInput").ap(),
        nc.dram_tensor("wvp", [C, D2], F32, kind="ExternalInput").ap(),
        nc.dram_tensor("bq", [D, 1], F32, kind="ExternalInput").ap(),
        nc.dram_tensor("bvp", [D2, 1], F32, kind="ExternalInput").ap(),
        nc.dram_tensor("out", [SQ, D], F32, kind="ExternalOutput").ap(),
    )
    with tile.TileContext(nc) as tc:
        for _ in range(reps):
            _emit(nc, tc, aps)
    nc.compile()
    return nc


def get_nc():
    if "nc" not in _CACHE:
        _CACHE["nc"] = _build()
    return _CACHE["nc"]


def make_in_maps(query, key_, value, Wq, bq, Wk, bk, Wv, bv):
    query, key_, value, Wq, bq, Wk, bk, Wv, bv = (
        np.asarray(a, dtype=np.float32)
        for a in (query, key_, value, Wq, bq, Wk, bk, Wv, bv)
    )
    wvp = np.concatenate([Wv, np.zeros((C, 2), np.float32)], axis=1)
    bvp = np.concatenate([bv, np.asarray([1.0, 0.0], np.float32)])[:, None]
    shared = {
        "wq": np.ascontiguousarray(Wq),
        "wk": np.ascontiguousarray(Wk),
        "wvp": np.ascontiguousarray(wvp),
        "bq": np.ascontiguousarray(bq[:, None]),
        "bvp": np.ascontiguousarray(bvp),
    }
    # host-side layout prep (cast + transpose only): k^T/v^T once per batch,
    # shared by the two cores that split the batch's queries
    ktb = [np.ascontiguousarray(key_[b].astype(BF_NP).T) for b in range(B)]
    vtb = [np.ascontiguousarray(value[b].astype(BF_NP).T) for b in range(B)]
    in_maps = []
    for c in range(N_CORES):
        b, h = divmod(c, 2)
        sl = slice(h * SQ, (h + 1) * SQ)
        in_maps.append(
            {
                "qt": np.ascontiguousarray(query[b, sl, :].astype(BF_NP).T),
                "kt": ktb[b],
                "vt": vtb[b],
                **shared,
            }
        )
    return in_maps


def assemble(results):
    out = np.empty((B, S, D), np.float32)
    for c in range(N_CORES):
        b, h = divmod(c, 2)
        out[b, h * SQ : (h + 1) * SQ, :] = results[c]["out"]
    return out


def kernel(query=None, key_=None, value=None, Wq=None, bq=None, Wk=None,
           bk=None, Wv=None, bv=None, key=None, **_):
    if key_ is None:
        key_ = key          # spec names this input "key"; reference uses "key_"
    nc = get_nc()
    in_maps = make_in_maps(query, key_, value, Wq, bq, Wk, bk, Wv, bv)
    res = run_bass_kernel_spmd(nc, in_maps, list(range(N_CORES)))
    return assemble(res.results)
